# revision 54
# baseline (speedup 1.0000x reference)
"""3-layer GAT (gnn_message_passing) on 8 Trainium2 NeuronCores.

Strategy: nodes sharded by dst octant.  Per layer:
  - node-parallel matmul phase computes Z=[h|es] rows for the core's 6250
    nodes (ed kept in SBUF tiles, never written to DRAM)
  - AllGather of the [h|es] table (bf16 rows, 1152B) to every core
  - dst-tile aggregation: dma_gather edge rows by src id; per-edge ed is
    broadcast from the local ed tile with a mskT matmul (mskT = PE
    transpose of the one-hot dst mask); logits exp(lrelu(es+ed))
    (segment-max-free softmax); selection-mask matmuls accumulate
    numerator+denominator directly in PSUM across all chunks of a dst
    tile; self-loops are handled by a sequential local-table read with an
    identity mask chunk; epilogue normalizes + bias (+relu) and
    transposes into the next layer's matmul input.
Edge structure (sorted by dst, tiled, A/B int16-index split, padding) is
precomputed on host; dummy table row 0 carries es=-1e9 so padded slots
get exp(-inf)=0 weight.
"""
import os
import sys

sys.path.insert(0, "/opt/trn_rl_repo")

import numpy as np
import ml_dtypes

BF16NP = ml_dtypes.bfloat16
NEG_BIG = -1.0e9

MAX_WAITS = 1


def _split_multiwait(nc):
    """walrus in this env rejects >1 sync-wait per instruction: split excess
    waits onto same-engine NoOps."""
    import concourse.mybir as mybir
    for _name, bbb in nc.bb_map.items():
        il = bbb.bb.instructions
        new = []
        changed = False
        for inst in il:
            si = getattr(inst, "sync_info", None)
            ow = list(si.on_wait) if si is not None and si.on_wait else []
            if len(ow) > MAX_WAITS:
                excess, keep = ow[:-MAX_WAITS], ow[-MAX_WAITS:]
                for j, w in enumerate(excess):
                    new.append(mybir.InstNoOp(
                        name=f"{inst.name}_sw{j}",
                        engine=inst.engine,
                        bass_nofuse=True,
                        sync_info=mybir.SyncInfo(on_wait=[w], on_update=[]),
                    ))
                inst.sync_info = mybir.SyncInfo(
                    on_wait=keep, on_update=list(si.on_update))
                changed = True
            new.append(inst)
        if changed:
            bbb.bb.instructions = new


# --------------------------------------------------------------------------
# host-side edge preprocessing
# --------------------------------------------------------------------------
def wrap16_block(vals, num):
    """dma_gather index layout: element i -> [i%16, i//16]; [16, num//16]."""
    a = np.zeros((16, num // 16), dtype=np.int16)
    v = np.asarray(vals, dtype=np.int64)
    i = np.arange(len(v))
    a[i % 16, i // 16] = v.astype(np.int16)
    return a


def table_perm(N, P, R, bnds):
    """Chunk-major table row permutation: node (p, r) with r in local chunk k
    (local rows [bnds[k]*128, bnds[k+1]*128)) lands at
    1 + base[k] + p*rows_k + (r - ra_k), so each AllGather chunk output is a
    contiguous block of rows."""
    ra = np.array([min(b * 128, R) for b in bnds], np.int64)
    rows_k = ra[1:] - ra[:-1]
    base = np.concatenate([[0], np.cumsum(P * rows_k)])
    perm = np.zeros(N, np.int64)
    s = np.arange(N)
    p = s // R
    r = s % R
    k = np.searchsorted(ra, r, side="right") - 1
    perm = 1 + base[k] + p * rows_k[k] + (r - ra[k])
    return perm, base, rows_k


def build_edges(edge_index, N, P, R, TPC, ATH, perm):
    """Per-core gather index/metadata arrays.  Self-loops are NOT included
    (handled by the device-side identity chunk).

    ATH: A/B row-id threshold (permuted row id < ATH -> pass A).
    Layout: tiles processed in pairs g=(2g, 2g+1); slot space per pair:
    [t0A][t1A][t0B][t1B]; per-(tile,group) chunk counts shared across cores.
    """
    src = np.asarray(edge_index[0], np.int64)
    dst = np.asarray(edge_index[1], np.int64)
    keep = src != dst          # drop explicit self-edges; loop added on device
    src, dst = src[keep], dst[keep]
    owner = dst // R

    cores = []
    cntA = np.zeros((P, TPC), np.int64)
    cntB = np.zeros((P, TPC), np.int64)
    for p in range(P):
        sel = np.nonzero(owner == p)[0]
        d = (dst[sel] - p * R)
        s = src[sel]
        row = perm[s]                     # permuted table row id
        grp = (row >= ATH).astype(np.int64)   # 0=A, 1=B
        order = np.lexsort((grp, d))      # sort by (dst_local, group)
        d = d[order]; row = row[order]; grp = grp[order]
        t = d // 128
        for g in (0, 1):
            c = np.bincount(t[grp == g], minlength=TPC)
            (cntA if g == 0 else cntB)[p] = c
        cores.append((d, row, grp, t))

    nA = np.maximum((cntA.max(axis=0) + 127) // 128, 1)
    nB = np.maximum((cntB.max(axis=0) + 127) // 128, 1)

    npairs = (TPC + 1) // 2
    pair_meta = []  # (tiles, aseg, bseg, chunk0, napad, nbpad)
    chunk0 = 0
    for g in range(npairs):
        tiles = [2 * g] + ([2 * g + 1] if 2 * g + 1 < TPC else [])
        a = [int(nA[t]) for t in tiles]
        b = [int(nB[t]) for t in tiles]
        napad = ((sum(a) + 1) // 2) * 2
        nbpad = ((sum(b) + 1) // 2) * 2
        pair_meta.append((tiles, a, b, chunk0, napad, nbpad))
        chunk0 += sum(a) + sum(b)
    NCH = chunk0

    out = []
    for p in range(P):
        d, row, grp, t = cores[p]
        idxA_cols = []
        idxB_cols = []
        dstrow = np.zeros((128, NCH), np.float32)
        dstrow_i = np.zeros((NCH, 128), np.int64)
        c0 = 0
        for (tiles, a, b, _c0, _np1, _np2) in pair_meta:
            arows = []
            brows = []
            for ti, tt in enumerate(tiles):
                selA = np.nonzero((t == tt) & (grp == 0))[0]
                nslot = a[ti] * 128
                va = np.zeros(nslot, np.int64)        # pad -> dummy row 0
                va[:len(selA)] = row[selA]
                ra = np.zeros(nslot, np.int64)
                ra[:len(selA)] = d[selA] - tt * 128
                arows.append((va, ra))
            for ti, tt in enumerate(tiles):
                selB = np.nonzero((t == tt) & (grp == 1))[0]
                nslot = b[ti] * 128
                vb = np.full(nslot, N + 65, np.int64)  # pad -> trailing dummy
                vb[:len(selB)] = row[selB]
                rb = np.zeros(nslot, np.int64)
                rb[:len(selB)] = d[selB] - tt * 128
                brows.append((vb, rb))
            va_all = np.concatenate([x[0] for x in arows])
            vb_all = np.concatenate([x[0] for x in brows])
            ra_all = np.concatenate([x[1] for x in arows] +
                                    [x[1] for x in brows])

            def padto(v, nch, fill):
                tgt = ((nch + 1) // 2) * 2 * 128
                o = np.full(tgt, fill, np.int64)
                o[:len(v)] = v
                return o
            va_pad = padto(va_all, len(va_all) // 128, 0)
            vb_pad = padto(vb_all - ATH, len(vb_all) // 128, N + 65 - ATH)
            idxA_cols.append(wrap16_block(va_pad, len(va_pad)))
            idxB_cols.append(wrap16_block(vb_pad, len(vb_pad)))
            nch_pair = (len(va_all) + len(vb_all)) // 128
            dstrow[:, c0:c0 + nch_pair] = \
                ra_all.reshape(nch_pair, 128).T.astype(np.float32)
            dstrow_i[c0:c0 + nch_pair] = ra_all.reshape(nch_pair, 128)
            c0 += nch_pair
        idxA = np.tile(np.concatenate(idxA_cols, axis=1), (8, 1))
        idxB = np.tile(np.concatenate(idxB_cols, axis=1), (8, 1))
        # host-precomputed one-hot masks (bf16):
        #   mskT[d, c*128+s] = (dstrow[s, c] == d)   (dst-major, ed bcast)
        #   mskS[s, c*128+d] = (dstrow[s, c] == d)   (slot-major, scatter)
        mskT = np.zeros((128, NCH * 128), BF16NP)
        cc_ = np.arange(NCH * 128) // 128
        ss_ = np.arange(NCH * 128) % 128
        mskT[dstrow_i[cc_, ss_], np.arange(NCH * 128)] = 1
        mskS = np.zeros((128, NCH * 128), BF16NP)
        cols = cc_ * 128 + dstrow_i[cc_, ss_]
        mskS[ss_, cols] = 1  # row s, col c*128+dst(c, s)
        out.append({"idxA": idxA, "idxB": idxB, "dstrow": dstrow,
                    "mskT": mskT, "mskS": mskS})
    return out, pair_meta, NCH


# --------------------------------------------------------------------------
# device program
# --------------------------------------------------------------------------
def build_program(cfg, pair_meta, NCH):
    import concourse.bass as bass
    import concourse.mybir as mybir
    import concourse.tile as tile
    from concourse.library_config import mlp
    from concourse.masks import make_identity
    from concourse.tile_rust import add_dep_helper

    def _mi(x):
        return getattr(x, "ins", x)

    def dep(a, b, why):
        add_dep_helper(_mi(a), _mi(b), reason=why)

    F32 = mybir.dt.float32
    BF16 = mybir.dt.bfloat16
    I16 = mybir.dt.int16

    N, P, R, TPC = cfg["N"], cfg["P"], cfg["R"], cfg["TPC"]
    F_IN, HID, HEADS, OUT = cfg["F_IN"], cfg["HID"], cfg["HEADS"], cfg["OUT"]
    ATH = cfg["ATH"]
    HC = HID * HEADS
    IN2 = HC + F_IN
    TROW = cfg["TROW"]            # bf16 cols of big table row (h|es|pad)
    TROW3 = cfg["TROW3"]
    DGAP = 64                     # spill gap before trailing dummy
    NRT = N + DGAP + 2            # rows: dummy, N nodes, gap, dummy
    NTILE = TPC * 128

    CA = sum(m[4] for m in pair_meta)
    CB = sum(m[5] for m in pair_meta)

    nc = bass.Bass()

    ps = {}
    def par(name, shape, dt):
        ps[name] = nc.declare_dram_parameter(name, list(shape), dt,
                                             isOutput=False)
        return ps[name]

    xT = par("xT", [F_IN, R], BF16)
    Wm1 = par("Wm1", [F_IN, HC], BF16)
    Wa1 = par("Wa1", [F_IN, 2 * HEADS], BF16)
    Wm2 = par("Wm2", [IN2, HC], BF16)
    Wa2 = par("Wa2", [IN2, 2 * HEADS], BF16)
    Wm3 = par("Wm3", [IN2, OUT], BF16)
    Wa3 = par("Wa3", [IN2, 2], BF16)
    b1 = par("b1", [128, HC], F32)
    b2 = par("b2", [128, HC], F32)
    b3 = par("b3", [128, OUT], F32)
    idxA_p = par("idxA", [128, CA * 8], I16)
    idxB_p = par("idxB", [128, CB * 8], I16)
    dstrow_p = par("dstrow", [128, NCH], F32)
    mskT_p = par("mskT", [128, NCH * 128], BF16)
    iota_p = par("iota", [128, 128], F32)
    dum576 = par("dum576", [1, TROW], BF16)
    dum128 = par("dum128", [1, TROW3], BF16)
    out_ext = nc.declare_dram_parameter("out", [R, OUT], F32, isOutput=True)

    T1 = nc.dram_tensor("T1", [NRT, TROW], BF16, addr_space="Shared")
    T2 = nc.dram_tensor("T2", [NRT, TROW], BF16, addr_space="Shared")
    T3 = nc.dram_tensor("T3", [NRT, TROW3], BF16, addr_space="Shared")
    T1sh = nc.dram_tensor("T1sh", [R, TROW], BF16)
    T2sh = nc.dram_tensor("T2sh", [R, TROW], BF16)
    T3sh = nc.dram_tensor("T3sh", [R, TROW3], BF16)
    barr_i = nc.dram_tensor("barr_i", [4, 64], F32)
    barr_o = nc.dram_tensor("barr_o", [4, 64], F32, addr_space="Shared")

    nc.gpsimd.load_library(mlp)

    from contextlib import ExitStack
    _regstack = ExitStack()
    _regcache = {}

    def numreg(v):
        if v not in _regcache:
            r = _regstack.enter_context(nc.gpsimd.register(f"nidx{v}"))
            nc.gpsimd.reg_mov(r, v)
            _regcache[v] = r
        return _regcache[v]

    with tile.TileContext(nc) as tc:
        with (
            tc.tile_pool(name="const", bufs=1) as constp,
            tc.tile_pool(name="w", bufs=1) as wp,
            tc.tile_pool(name="xt", bufs=1) as xtp,
            tc.tile_pool(name="outT", bufs=1) as outTp,
            tc.tile_pool(name="ed", bufs=1) as edp,
            tc.tile_pool(name="mmz", bufs=2) as mmzp,
            tc.tile_pool(name="gA", bufs=2) as gAp,
            tc.tile_pool(name="gB", bufs=2) as gBp,
            tc.tile_pool(name="gS", bufs=2) as gSp,
            tc.tile_pool(name="seg", bufs=2) as segp,
            tc.tile_pool(name="mt", bufs=3) as mtp,
            tc.tile_pool(name="ep", bufs=2) as epp,
            tc.tile_pool(name="psA", bufs=3, space="PSUM") as psAp,
            tc.tile_pool(name="psD", bufs=2, space="PSUM") as psDp,
            tc.tile_pool(name="psE", bufs=2, space="PSUM") as psEp,
            tc.tile_pool(name="psT", bufs=1, space="PSUM") as psTp,
        ):
            # ---------- constants / resident data
            ident = constp.tile([128, 128], BF16, tag="ident")
            make_identity(nc, ident[:])
            iota_f = constp.tile([128, 128], F32, tag="iotaf")
            nc.sync.dma_start(out=iota_f[:], in_=iota_p[:])
            c02 = constp.tile([128, 1], F32, tag="c02")
            nc.vector.memset(c02[:], cfg["NEG_SLOPE"])

            idxA_sb = constp.tile([128, CA * 8], I16, tag="idxA")
            nc.sync.dma_start(out=idxA_sb[:], in_=idxA_p[:])
            idxB_sb = constp.tile([128, CB * 8], I16, tag="idxB")
            nc.sync.dma_start(out=idxB_sb[:], in_=idxB_p[:])
            dstrow_sb = constp.tile([128, NCH], F32, tag="dstrow")
            nc.sync.dma_start(out=dstrow_sb[:], in_=dstrow_p[:])

            bias_sb = {}
            for nm, p_, w_ in (("b1", b1, HC), ("b2", b2, HC), ("b3", b3, OUT)):
                bias_sb[nm] = constp.tile([128, w_], F32, tag=nm, name=nm)
                nc.sync.dma_start(out=bias_sb[nm][:], in_=p_[:])

            DB = N + DGAP + 1
            dummy_w = {}
            dummy_src = {}
            for T_, dum in ((T1, dum576), (T2, dum576), (T3, dum128)):
                i1 = nc.sync.dma_start(out=T_[0:1, :], in_=dum[:])
                i2 = nc.sync.dma_start(out=T_[DB:DB + 1, :], in_=dum[:])
                dummy_w[id(T_)] = [i1, i2]
                dummy_src[id(T_)] = dum

            def load_w(p_, rows, cols, tag):
                nchunks = (rows + 127) // 128
                tl = wp.tile([128, nchunks * cols], BF16, tag=tag, name=tag)
                for fc in range(nchunks):
                    r0 = fc * 128
                    vr = min(128, rows - r0)
                    nc.sync.dma_start(out=tl[:vr, fc * cols:(fc + 1) * cols],
                                      in_=p_[r0:r0 + vr, :])
                return tl

            Wm1_sb = load_w(Wm1, F_IN, HC, "Wm1")
            Wa1_sb = load_w(Wa1, F_IN, 2 * HEADS, "Wa1")
            Wm2_sb = load_w(Wm2, IN2, HC, "Wm2")
            Wa2_sb = load_w(Wa2, IN2, 2 * HEADS, "Wa2")
            Wm3_sb = load_w(Wm3, IN2, OUT, "Wm3")
            Wa3_sb = load_w(Wa3, IN2, 2, "Wa3")

            xT_sb = xtp.tile([128, (F_IN // 128) * R], BF16, tag="xT")
            for fc in range(F_IN // 128):
                nc.sync.dma_start(out=xT_sb[:, fc * R:(fc + 1) * R],
                                  in_=xT[fc * 128:(fc + 1) * 128, :])

            outT_sb = outTp.tile([128, (HC // 128) * NTILE], BF16, tag="outT")
            # per-layer local ed tiles: bf16 (matmul rhs) + f32 (self chunk)
            edb_sb = edp.tile([128, TPC * 8], BF16, tag="edb")
            edf_sb = edp.tile([128, TPC * 8], F32, tag="edf")

            # ---------- matmul phase (one tile)
            def mm_tile(layer, rt, tsh_w):
                if layer == 1:
                    nfc, Wm_sb, Wa_sb, Tsh, trow, hcols, nh = (
                        F_IN // 128, Wm1_sb, Wa1_sb, T1sh, TROW, HC, HEADS)
                elif layer == 2:
                    nfc, Wm_sb, Wa_sb, Tsh, trow, hcols, nh = (
                        IN2 // 128, Wm2_sb, Wa2_sb, T2sh, TROW, HC, HEADS)
                else:
                    nfc, Wm_sb, Wa_sb, Tsh, trow, hcols, nh = (
                        IN2 // 128, Wm3_sb, Wa3_sb, T3sh, TROW3, OUT, 1)
                acols = 2 * nh
                nxc = HC // 128

                r0 = rt * 128
                vr = min(128, R - r0)
                if vr <= 0:
                    return
                pm = psAp.tile([128, max(hcols, 8)], F32, tag="agg",
                               name="pm")
                pa = psDp.tile([128, 16], F32, tag="den", name="pa")
                for fc in range(nfc):
                    if layer == 1:
                        lhsT = xT_sb[:, fc * R + r0: fc * R + r0 + vr]
                    elif fc < nxc:
                        lhsT = outT_sb[:, fc * NTILE + r0:
                                       fc * NTILE + r0 + vr]
                    else:
                        fx = fc - nxc
                        lhsT = xT_sb[:, fx * R + r0: fx * R + r0 + vr]
                    nc.tensor.matmul(out=pm[:vr, :hcols], lhsT=lhsT,
                                     rhs=Wm_sb[:, fc * hcols:(fc + 1) * hcols],
                                     start=(fc == 0), stop=(fc == nfc - 1))
                    nc.tensor.matmul(out=pa[:vr, :acols], lhsT=lhsT,
                                     rhs=Wa_sb[:, fc * acols:(fc + 1) * acols],
                                     start=(fc == 0), stop=(fc == nfc - 1))
                zrow = mmzp.tile([128, TROW], BF16, tag="zrow")
                nc.vector.memset(zrow[:, hcols + 2 * nh:trow], 0.0)
                nc.vector.tensor_copy(out=zrow[:vr, :hcols],
                                      in_=pm[:vr, :hcols])
                nc.vector.tensor_copy(
                    out=zrow[:vr, hcols:hcols + 2 * nh].bitcast(F32),
                    in_=pa[:vr, 0:nh])
                nc.vector.tensor_copy(out=edb_sb[:vr, rt * 8:rt * 8 + nh],
                                      in_=pa[:vr, nh:2 * nh])
                nc.vector.tensor_copy(out=edf_sb[:vr, rt * 8:rt * 8 + nh],
                                      in_=pa[:vr, nh:2 * nh])
                tsh_w.append(nc.sync.dma_start(out=Tsh[r0:r0 + vr, :],
                                               in_=zrow[:vr, :trow]))

            # per-layer chunked AllGather state
            def ag_chunk(layer, k, Tsh_, T_, tsh_w, ccs):
                bnds = cfg["AG_BNDS"]
                base = cfg["AG_BASE"]
                ta, tb = bnds[k], bnds[k + 1]
                if tb <= ta:
                    return
                ra, rb = ta * 128, min(tb * 128, R)
                o0 = 1 + int(base[k])
                cc = nc.gpsimd.collective_compute(
                    "AllGather",
                    mybir.AluOpType.bypass,
                    replica_groups=[list(range(P))],
                    ins=[Tsh_[ra:rb, :]],
                    outs=[T_[o0:o0 + P * (rb - ra), :]],
                )
                for t in range(ta, tb):
                    dep(cc, tsh_w[t], "AG chunk reads shard rows")
                ccs.append(cc)

            last_gather = [None]

            # ---------- aggregation phase
            def agg_phase(layer, ccs, Tsh_, tsh_w, after_tile=None):
                pass_T = {1: T1, 2: T2, 3: T3}[layer]
                # cross-core barrier: AllReduce completes only once every
                # core's CC stream (incl. its AG chunk sends) has drained,
                # so gathers can't read rows whose remote writes are in
                # flight (exposed by profiling skew between cores).
                bar = nc.gpsimd.collective_compute(
                    "AllReduce",
                    mybir.AluOpType.add,
                    replica_groups=[list(range(P))],
                    ins=[barr_i[layer:layer + 1, :]],
                    outs=[barr_o[layer:layer + 1, :]],
                )
                for cc in ccs:
                    dep(bar, cc, "barrier after AG chunks")
                ccs = ccs + [bar]
                # refresh dummy rows after AG chunks (guards against any
                # collective overrun clobbering the trailing dummy row)
                dre = nc.sync.dma_start(out=pass_T[DB:DB + 1, :],
                                        in_=dummy_src[id(pass_T)][:])
                for cc in ccs:
                    dep(dre, cc, "dummy refresh after AG chunks")
                dummy_w[id(pass_T)] = dummy_w[id(pass_T)] + [dre]
                if layer == 3:
                    T_, trow, hcols, nh = T3, TROW3, OUT, 1
                    bias = bias_sb["b3"]
                else:
                    T_, trow, hcols, nh = (T1 if layer == 1 else T2), TROW, HC, HEADS
                    bias = bias_sb["b1"] if layer == 1 else bias_sb["b2"]

                offA = offB = 0
                for (tiles, aseg, bseg, c0, napad, nbpad) in pair_meta:
                    ntl = len(tiles)
                    bufA = gAp.tile([128, cfg["MAXA"] * trow], BF16, tag="bufA")
                    bufB = gBp.tile([128, cfg["MAXB"] * trow], BF16, tag="bufB")
                    SUBG = 8
                    gAs, gBs = [], []
                    for cs in range(0, napad, SUBG):
                        ck = min(SUBG, napad - cs)
                        gAs.append(nc.gpsimd.dma_gather(
                            bufA[:, cs * trow:(cs + ck) * trow].rearrange(
                                "p (c w) -> p c w", w=trow),
                            T_[:], idxA_sb[:, offA + cs * 8:offA + (cs + ck) * 8],
                            ck * 128, numreg(ck * 128), trow))
                    for cs in range(0, nbpad, SUBG):
                        ck = min(SUBG, nbpad - cs)
                        gBs.append(nc.gpsimd.dma_gather(
                            bufB[:, cs * trow:(cs + ck) * trow].rearrange(
                                "p (c w) -> p c w", w=trow),
                            T_[ATH:, :],
                            idxB_sb[:, offB + cs * 8:offB + (cs + ck) * 8],
                            ck * 128, numreg(ck * 128), trow))
                    for g_ in gAs + gBs:
                        for cc in ccs:
                            dep(g_, cc, "gather reads allgathered table")
                        for d_ in dummy_w[id(T_)]:
                            dep(g_, d_, "gather reads dummy rows")
                        last_gather[0] = g_
                    offA += napad * 8; offB += nbpad * 8

                    # self rows (own shard, sequential read; no AG dep)
                    selfr = gSp.tile([128, 2 * trow], BF16, tag="selfr")
                    for ti, tt in enumerate(tiles):
                        r0l = tt * 128
                        vr = min(128, R - r0l)
                        sr = nc.sync.dma_start(
                            out=selfr[:vr, ti * trow:(ti + 1) * trow],
                            in_=Tsh_[r0l:r0l + vr, :])
                        dep(sr, tsh_w[tt], "self rows read own shard write")

                    # psum accumulators per tile
                    pag = [psAp.tile([128, max(hcols, 8)], F32, tag="agg",
                                     name=f"pag{_i}") for _i in range(ntl)]
                    pde = [psDp.tile([128, 8], F32, tag="den",
                                     name=f"pde{_i}") for _i in range(ntl)]
                    started = [False] * ntl

                    # segments: (tile_i, buf, bufc0, nchunks)
                    segs = []
                    bc = 0
                    for ti in range(ntl):
                        segs.append((ti, bufA, bc, aseg[ti]))
                        bc += aseg[ti]
                    bc = 0
                    for ti in range(ntl):
                        segs.append((ti, bufB, bc, bseg[ti]))
                        bc += bseg[ti]

                    pc = 0
                    for (ti, buf, bc0, nck) in segs:
                        if nck == 0:
                            continue
                        bv = buf[:, : (bc0 + nck) * trow].rearrange(
                            "p (c w) -> p c w", w=trow)
                        # one-hot dst masks for the segment
                        msk = segp.tile([128, cfg["MAXSEG"] * 128], BF16,
                                        tag="msk")
                        nc.vector.tensor_tensor(
                            out=msk[:, :nck * 128].rearrange(
                                "p (c r) -> p c r", c=nck),
                            in0=dstrow_sb[:, c0 + pc:c0 + pc + nck, None]
                                .to_broadcast([128, nck, 128]),
                            in1=iota_f[:, None, :]
                                .to_broadcast([128, nck, 128]),
                            op=mybir.AluOpType.is_equal)
                        # ed broadcast matmul with host-precomputed mskT
                        mtT = mtp.tile([128, cfg["MAXSEG"] * 128], BF16,
                                       tag="mtT", name="mtT")
                        nc.scalar.dma_start(
                            out=mtT[:, :nck * 128],
                            in_=mskT_p[:, (c0 + pc) * 128:
                                       (c0 + pc + nck) * 128])
                        pse = psEp.tile([128, cfg["MAXSEG"] * 8], F32,
                                        tag="pse", name="pse")
                        for j in range(nck):
                            nc.tensor.matmul(
                                out=pse[:, j * 8:j * 8 + nh],
                                lhsT=mtT[:, j * 128:(j + 1) * 128],
                                rhs=edb_sb[:, tiles[ti] * 8:tiles[ti] * 8 + nh],
                                start=True, stop=True,
                                skip_group_check=True)
                        # logits
                        es_ap = bv[:, bc0:bc0 + nck,
                                   hcols:hcols + 2 * nh].bitcast(F32)
                        et = segp.tile([128, cfg["MAXSEG"] * 8], F32, tag="et")
                        nc.vector.tensor_tensor(
                            out=et[:, :nck * 8].rearrange(
                                "p (c h) -> p c h", h=8)[:, :, :nh],
                            in0=es_ap,
                            in1=pse[:, :nck * 8].rearrange(
                                "p (c h) -> p c h", h=8)[:, :, :nh],
                            op=mybir.AluOpType.add)
                        etl = segp.tile([128, cfg["MAXSEG"] * 8], F32,
                                        tag="etl")
                        def _v8(t):
                            return t[:, :nck * 8].rearrange(
                                "p (c h) -> p c h", h=8)[:, :, :nh]
                        nc.vector.tensor_tensor(
                            out=_v8(etl), in0=_v8(et),
                            in1=c02[:, 0:1, None].to_broadcast([128, nck, nh]),
                            op=mybir.AluOpType.mult)
                        nc.vector.tensor_tensor(
                            out=_v8(et), in0=_v8(et),
                            in1=_v8(etl), op=mybir.AluOpType.max)
                        ex = segp.tile([128, cfg["MAXSEG"] * 8], BF16,
                                       tag="ex")
                        nc.scalar.activation(
                            out=_v8(ex), in_=_v8(et),
                            func=mybir.ActivationFunctionType.Exp)
                        # scaled messages
                        mp_ = segp.tile([128, cfg["MAXSEG"] * hcols], BF16,
                                        tag="mp")
                        nc.vector.tensor_tensor(
                            out=mp_[:, :nck * hcols].rearrange(
                                "p (c h k) -> p c h k", c=nck, h=nh),
                            in0=bv[:, bc0:bc0 + nck, 0:hcols].rearrange(
                                "p c (h k) -> p c h k", h=nh),
                            in1=ex[:, :nck * 8].rearrange(
                                "p (c h) -> p c h", h=8)[:, :, :nh, None]
                                .to_broadcast([128, nck, nh, hcols // nh]),
                            op=mybir.AluOpType.mult)
                        for j in range(nck):
                            nc.tensor.matmul(
                                out=pag[ti][:, :hcols],
                                lhsT=msk[:, j * 128:(j + 1) * 128],
                                rhs=mp_[:, j * hcols:(j + 1) * hcols],
                                start=not started[ti], stop=False,
                                skip_group_check=True)
                            nc.tensor.matmul(
                                out=pde[ti][:, :nh],
                                lhsT=msk[:, j * 128:(j + 1) * 128],
                                rhs=ex[:, j * 8:j * 8 + nh],
                                start=not started[ti], stop=False,
                                skip_group_check=True)
                            started[ti] = True
                        pc += nck

                    # self chunk + epilogue per tile
                    for ti, tt in enumerate(tiles):
                        vr = min(128, R - tt * 128)
                        sv = selfr[:, ti * trow:(ti + 1) * trow]
                        es_s = sv[:, hcols:hcols + 2 * nh].bitcast(F32)
                        ets = mtp.tile([128, 8], F32, tag="ets", name="ets")
                        nc.vector.tensor_tensor(
                            out=ets[:, :nh], in0=es_s,
                            in1=edf_sb[:, tt * 8:tt * 8 + nh],
                            op=mybir.AluOpType.add)
                        etls = mtp.tile([128, 8], F32, tag="etls",
                                        name="etls")
                        nc.vector.tensor_tensor(
                            out=etls[:, :nh], in0=ets[:, :nh],
                            in1=c02[:, 0:1].to_broadcast([128, nh]),
                            op=mybir.AluOpType.mult)
                        nc.vector.tensor_tensor(
                            out=ets[:, :nh], in0=ets[:, :nh],
                            in1=etls[:, :nh], op=mybir.AluOpType.max)
                        exs = mtp.tile([128, 8], BF16, tag="exs", name="exs")
                        nc.scalar.activation(
                            out=exs[:, :nh], in_=ets[:, :nh],
                            func=mybir.ActivationFunctionType.Exp)
                        mps = mtp.tile([128, hcols], BF16, tag="mps",
                                       name="mps")
                        nc.vector.tensor_tensor(
                            out=mps[:].rearrange("p (h k) -> p h k", h=nh),
                            in0=sv[:, 0:hcols].rearrange(
                                "p (h k) -> p h k", h=nh),
                            in1=exs[:, :nh, None].to_broadcast(
                                [128, nh, hcols // nh]),
                            op=mybir.AluOpType.mult)
                        nc.tensor.matmul(
                            out=pag[ti][:, :hcols], lhsT=ident[:],
                            rhs=mps[:], start=not started[ti], stop=True,
                            skip_group_check=True)
                        nc.tensor.matmul(
                            out=pde[ti][:, :nh], lhsT=ident[:],
                            rhs=exs[:, :nh], start=not started[ti],
                            stop=True, skip_group_check=True)

                        den = epp.tile([128, 8], F32, tag="den_sb")
                        nc.vector.reciprocal(out=den[:, :nh],
                                             in_=pde[ti][:, :nh])
                        o1 = epp.tile([128, max(hcols, 8)], F32, tag="o1")
                        nc.vector.tensor_tensor(
                            out=o1[:, :hcols].rearrange(
                                "p (h k) -> p h k", h=nh),
                            in0=pag[ti][:, :hcols].rearrange(
                                "p (h k) -> p h k", h=nh),
                            in1=den[:, :nh, None].to_broadcast(
                                [128, nh, hcols // nh]),
                            op=mybir.AluOpType.mult)
                        nc.vector.tensor_tensor(
                            out=o1[:, :hcols], in0=o1[:, :hcols],
                            in1=bias[:, :],
                            op=mybir.AluOpType.add)
                        r0 = tt * 128
                        if layer != 3:
                            ob = epp.tile([128, hcols], BF16, tag="ob")
                            nc.scalar.activation(
                                out=ob[:, :], in_=o1[:, :hcols],
                                func=mybir.ActivationFunctionType.Relu)
                            for q in range(hcols // 128):
                                pt2 = psTp.tile([128, 128], BF16, tag="pt2",
                                                name="pt2")
                                nc.tensor.transpose(
                                    out=pt2[:, :vr],
                                    in_=ob[:vr, q * 128:(q + 1) * 128],
                                    identity=ident[:vr, :vr])
                                nc.vector.tensor_copy(
                                    out=outT_sb[:, q * NTILE + r0:
                                                q * NTILE + r0 + vr],
                                    in_=pt2[:, :vr])
                        else:
                            mx = epp.tile([128, 1], F32, tag="mx")
                            nc.vector.tensor_reduce(
                                out=mx[:], in_=o1[:, :hcols],
                                op=mybir.AluOpType.max,
                                axis=mybir.AxisListType.X)
                            zc = epp.tile([128, hcols], F32, tag="zc")
                            nc.vector.tensor_scalar(
                                out=zc[:], in0=o1[:, :hcols], scalar1=mx[:],
                                scalar2=None,
                                op0=mybir.AluOpType.subtract)
                            ex3 = epp.tile([128, hcols], F32, tag="ex3")
                            s3 = epp.tile([128, 1], F32, tag="s3")
                            nc.scalar.activation(
                                out=ex3[:], in_=zc[:],
                                func=mybir.ActivationFunctionType.Exp,
                                accum_out=s3[:])
                            ln3 = epp.tile([128, 1], F32, tag="ln3")
                            nc.scalar.activation(
                                out=ln3[:], in_=s3[:],
                                func=mybir.ActivationFunctionType.Ln)
                            res = epp.tile([128, hcols], F32, tag="res")
                            nc.vector.tensor_scalar(
                                out=res[:], in0=zc[:], scalar1=ln3[:],
                                scalar2=None,
                                op0=mybir.AluOpType.subtract)
                            nc.sync.dma_start(out=out_ext[r0:r0 + vr, :],
                                              in_=res[:vr, :])
                        if after_tile is not None:
                            after_tile(tt)

            # ---------- the three layers
            # layer k+1's matmul tiles are emitted inside layer k's
            # aggregation (after each tile's epilogue) so the tensor work
            # overlaps the gather-bound agg phase.  L1's AllGather is
            # chunked to overlap L1's matmul phase; L2/L3 AllGathers are
            # monolithic and serialize after the previous agg (their tsh
            # deps transitively follow every gather of the previous
            # layer), keeping collective DMA and gather DMA disjoint.
            bnds = cfg["AG_BNDS"]
            chunk_end = {bnds[k + 1] - 1: k for k in range(len(bnds) - 1)
                         if bnds[k + 1] > bnds[k]}
            tshs = {1: [], 2: [], 3: []}
            ccss = {1: [], 2: [], 3: []}
            tabs = {1: (T1sh, T1), 2: (T2sh, T2), 3: (T3sh, T3)}

            for rt in range(TPC):
                mm_tile(1, rt, tshs[1])
                if rt in chunk_end:
                    ag_chunk(1, chunk_end[rt], T1sh, T1, tshs[1], ccss[1])

            def mk_after(nl):
                def after(tt):
                    mm_tile(nl, tt, tshs[nl])
                return after

            def ag_serial(nl):
                # chunk-major AG chunks, serialized after the previous
                # layer's last gather so CC DMA never overlaps gather DMA
                Tsh_n, T_n = tabs[nl]
                lg = last_gather[0]
                for k in range(len(bnds) - 1):
                    if bnds[k + 1] <= bnds[k]:
                        continue
                    ag_chunk(nl, k, Tsh_n, T_n, tshs[nl], ccss[nl])
                    if lg is not None:
                        dep(ccss[nl][-1], lg,
                            "AG serialized after prev-layer gathers")

            def standalone(nl):
                for rt in range(TPC):
                    mm_tile(nl, rt, tshs[nl])
                    if rt in chunk_end:
                        ag_chunk(nl, chunk_end[rt], tabs[nl][0], tabs[nl][1],
                                 tshs[nl], ccss[nl])

            agg_phase(1, ccss[1], T1sh, tshs[1])
            standalone(2)
            agg_phase(2, ccss[2], T2sh, tshs[2])
            standalone(3)
            agg_phase(3, ccss[3], T3sh, tshs[3])

    _regstack.close()
    from concourse.library_overlay import lower_extended_insts
    lower_extended_insts(nc)
    return nc


# --------------------------------------------------------------------------
# host wrapper
# --------------------------------------------------------------------------
def _prep_inputs(inputs, cfg):
    N, P, R, TPC = cfg["N"], cfg["P"], cfg["R"], cfg["TPC"]
    HEADS, HID, OUT, F_IN = cfg["HEADS"], cfg["HID"], cfg["OUT"], cfg["F_IN"]
    HC = HEADS * HID

    x = np.asarray(inputs["x"], np.float32)
    edge_index = np.asarray(inputs["edge_index"], np.int64)

    nchunks = min(6, TPC)
    bnds = [TPC * k // nchunks for k in range(nchunks + 1)]
    perm, base, _rows_k = table_perm(N, P, R, bnds)
    cfg["AG_BNDS"] = bnds
    cfg["AG_BASE"] = base
    shards, pair_meta, NCH = build_edges(edge_index, N, P, R, TPC, cfg["ATH"],
                                         perm)

    def fold(W, a_s, a_d, heads, ch):
        F = W.shape[0]
        Wr = W.reshape(F, heads, ch)
        Wa = np.zeros((F, 2 * heads), np.float32)
        for h in range(heads):
            Wa[:, h] = Wr[:, h] @ a_s[h]
            Wa[:, heads + h] = Wr[:, h] @ a_d[h]
        return Wa

    w1 = np.asarray(inputs["w1"], np.float32)
    w2 = np.asarray(inputs["w2"], np.float32)
    w3 = np.asarray(inputs["w3"], np.float32)
    Wa1 = fold(w1, np.asarray(inputs["a1s"]), np.asarray(inputs["a1d"]),
               HEADS, HID)
    Wa2 = fold(w2, np.asarray(inputs["a2s"]), np.asarray(inputs["a2d"]),
               HEADS, HID)
    Wa3 = fold(w3, np.asarray(inputs["a3s"]), np.asarray(inputs["a3d"]),
               1, OUT)

    dum576 = np.zeros((1, cfg["TROW"]), BF16NP)
    dum576.view(np.uint8)[0, 2 * HC:2 * HC + HEADS * 4] = \
        np.full(HEADS, NEG_BIG, np.float32).view(np.uint8)
    dum128 = np.zeros((1, cfg["TROW3"]), BF16NP)
    dum128.view(np.uint8)[0, 2 * OUT:2 * OUT + 4] = \
        np.frombuffer(np.float32(NEG_BIG).tobytes(), np.uint8)

    common = {
        "Wm1": w1.astype(BF16NP), "Wa1": Wa1.astype(BF16NP),
        "Wm2": w2.astype(BF16NP), "Wa2": Wa2.astype(BF16NP),
        "Wm3": w3.astype(BF16NP), "Wa3": Wa3.astype(BF16NP),
        "b1": np.tile(np.asarray(inputs["b1"], np.float32).reshape(1, HC),
                      (128, 1)),
        "b2": np.tile(np.asarray(inputs["b2"], np.float32).reshape(1, HC),
                      (128, 1)),
        "b3": np.tile(np.asarray(inputs["b3"], np.float32).reshape(1, OUT),
                      (128, 1)),
        "dum576": dum576, "dum128": dum128,
        "iota": np.tile(np.arange(128, dtype=np.float32), (128, 1)),
    }
    in_maps = []
    for p in range(P):
        m = dict(common)
        m["xT"] = np.ascontiguousarray(
            x[p * R:(p + 1) * R, :].T).astype(BF16NP)
        m["idxA"] = shards[p]["idxA"]
        m["idxB"] = shards[p]["idxB"]
        m["dstrow"] = shards[p]["dstrow"]
        m["mskT"] = shards[p]["mskT"]
        in_maps.append(m)

    maxa = max(m[4] for m in pair_meta)
    maxb = max(m[5] for m in pair_meta)
    cfg["MAXA"], cfg["MAXB"] = maxa, maxb
    cfg["MAXSEG"] = max(max(a + b) for (_, a, b, _c, _1, _2) in pair_meta)
    return in_maps, pair_meta, NCH


def default_cfg():
    return dict(N=50000, P=8, R=6250, TPC=49, F_IN=256, HID=64, HEADS=8,
                OUT=16, ATH=32768, TROW=640, TROW3=128, NEG_SLOPE=0.2)


def run_with_cfg(cfg, inputs):
    in_maps, pair_meta, NCH = _prep_inputs(inputs, cfg)
    nc = build_program(cfg, pair_meta, NCH)

    _split_multiwait(nc)
    from concourse.bass_utils import run_bass_kernel_spmd
    trace = bool(os.environ.get("GNN_TRACE"))
    if trace:
        sys.path.insert(0, "/root/problem/work")
        import axonhook  # noqa
    res = run_bass_kernel_spmd(nc, in_maps, list(range(cfg["P"])),
                               trace=trace)
    if trace:
        kernel.last_exec_ns = res.exec_time_ns
    out = np.concatenate([res.results[p]["out"] for p in range(cfg["P"])],
                         axis=0)
    return out.astype(np.float32)


def kernel(**inputs):
    return run_with_cfg(default_cfg(), inputs)


# revision 55
# speedup vs baseline: 1.1371x; 1.1371x over previous
"""3-layer GAT (gnn_message_passing) on 8 Trainium2 NeuronCores.

Strategy: nodes sharded by dst octant.  Per layer:
  - node-parallel matmul phase computes Z=[h|es] rows for the core's 6250
    nodes (ed kept in SBUF tiles, never written to DRAM)
  - AllGather of the [h|es] table (bf16 rows, 1152B) to every core
  - dst-tile aggregation: dma_gather edge rows by src id; per-edge ed is
    broadcast from the local ed tile with a mskT matmul (mskT = PE
    transpose of the one-hot dst mask); logits exp(lrelu(es+ed))
    (segment-max-free softmax); selection-mask matmuls accumulate
    numerator+denominator directly in PSUM across all chunks of a dst
    tile; self-loops are handled by a sequential local-table read with an
    identity mask chunk; epilogue normalizes + bias (+relu) and
    transposes into the next layer's matmul input.
Edge structure (sorted by dst, tiled, A/B int16-index split, padding) is
precomputed on host; dummy table row 0 carries es=-1e9 so padded slots
get exp(-inf)=0 weight.
"""
import os
import sys

sys.path.insert(0, "/opt/trn_rl_repo")

import numpy as np
import ml_dtypes

BF16NP = ml_dtypes.bfloat16
NEG_BIG = -1.0e9

MAX_WAITS = 1


def _split_multiwait(nc):
    """walrus in this env rejects >1 sync-wait per instruction: split excess
    waits onto same-engine NoOps."""
    import concourse.mybir as mybir
    for _name, bbb in nc.bb_map.items():
        il = bbb.bb.instructions
        new = []
        changed = False
        for inst in il:
            si = getattr(inst, "sync_info", None)
            ow = list(si.on_wait) if si is not None and si.on_wait else []
            if len(ow) > MAX_WAITS:
                excess, keep = ow[:-MAX_WAITS], ow[-MAX_WAITS:]
                for j, w in enumerate(excess):
                    new.append(mybir.InstNoOp(
                        name=f"{inst.name}_sw{j}",
                        engine=inst.engine,
                        bass_nofuse=True,
                        sync_info=mybir.SyncInfo(on_wait=[w], on_update=[]),
                    ))
                inst.sync_info = mybir.SyncInfo(
                    on_wait=keep, on_update=list(si.on_update))
                changed = True
            new.append(inst)
        if changed:
            bbb.bb.instructions = new


# --------------------------------------------------------------------------
# host-side edge preprocessing
# --------------------------------------------------------------------------
def wrap16_block(vals, num):
    """dma_gather index layout: element i -> [i%16, i//16]; [16, num//16]."""
    a = np.zeros((16, num // 16), dtype=np.int16)
    v = np.asarray(vals, dtype=np.int64)
    i = np.arange(len(v))
    a[i % 16, i // 16] = v.astype(np.int16)
    return a


def table_perm(N, P, R, bnds):
    """Chunk-major table row permutation: node (p, r) with r in local chunk k
    (local rows [bnds[k]*128, bnds[k+1]*128)) lands at
    1 + base[k] + p*rows_k + (r - ra_k), so each AllGather chunk output is a
    contiguous block of rows."""
    ra = np.array([min(b * 128, R) for b in bnds], np.int64)
    rows_k = ra[1:] - ra[:-1]
    base = np.concatenate([[0], np.cumsum(P * rows_k)])
    perm = np.zeros(N, np.int64)
    s = np.arange(N)
    p = s // R
    r = s % R
    k = np.searchsorted(ra, r, side="right") - 1
    perm = 1 + base[k] + p * rows_k[k] + (r - ra[k])
    return perm, base, rows_k


def build_edges(edge_index, N, P, R, TPC, ATH, perm):
    """Per-core gather index/metadata arrays.  Self-loops are NOT included
    (handled by the device-side identity chunk).

    ATH: A/B row-id threshold (permuted row id < ATH -> pass A).
    Layout: tiles processed in pairs g=(2g, 2g+1); slot space per pair:
    [t0A][t1A][t0B][t1B]; per-(tile,group) chunk counts shared across cores.
    """
    src = np.asarray(edge_index[0], np.int64)
    dst = np.asarray(edge_index[1], np.int64)
    keep = src != dst          # drop explicit self-edges; loop added on device
    src, dst = src[keep], dst[keep]
    owner = dst // R

    cores = []
    cntA = np.zeros((P, TPC), np.int64)
    cntB = np.zeros((P, TPC), np.int64)
    for p in range(P):
        sel = np.nonzero(owner == p)[0]
        d = (dst[sel] - p * R)
        s = src[sel]
        row = perm[s]                     # permuted table row id
        grp = (row >= ATH).astype(np.int64)   # 0=A, 1=B
        order = np.lexsort((grp, d))      # sort by (dst_local, group)
        d = d[order]; row = row[order]; grp = grp[order]
        t = d // 128
        for g in (0, 1):
            c = np.bincount(t[grp == g], minlength=TPC)
            (cntA if g == 0 else cntB)[p] = c
        cores.append((d, row, grp, t))

    nA = np.maximum((cntA.max(axis=0) + 127) // 128, 1)
    nB = np.maximum((cntB.max(axis=0) + 127) // 128, 1)

    npairs = (TPC + 1) // 2
    pair_meta = []  # (tiles, aseg, bseg, chunk0, napad, nbpad)
    chunk0 = 0
    for g in range(npairs):
        tiles = [2 * g] + ([2 * g + 1] if 2 * g + 1 < TPC else [])
        a = [int(nA[t]) for t in tiles]
        b = [int(nB[t]) for t in tiles]
        napad = ((sum(a) + 1) // 2) * 2
        nbpad = ((sum(b) + 1) // 2) * 2
        pair_meta.append((tiles, a, b, chunk0, napad, nbpad))
        chunk0 += sum(a) + sum(b)
    NCH = chunk0

    out = []
    for p in range(P):
        d, row, grp, t = cores[p]
        idxA_cols = []
        idxB_cols = []
        dstrow = np.zeros((128, NCH), np.float32)
        dstrow_i = np.zeros((NCH, 128), np.int64)
        c0 = 0
        for (tiles, a, b, _c0, _np1, _np2) in pair_meta:
            arows = []
            brows = []
            for ti, tt in enumerate(tiles):
                selA = np.nonzero((t == tt) & (grp == 0))[0]
                nslot = a[ti] * 128
                va = np.zeros(nslot, np.int64)        # pad -> dummy row 0
                va[:len(selA)] = row[selA]
                ra = np.zeros(nslot, np.int64)
                ra[:len(selA)] = d[selA] - tt * 128
                arows.append((va, ra))
            for ti, tt in enumerate(tiles):
                selB = np.nonzero((t == tt) & (grp == 1))[0]
                nslot = b[ti] * 128
                vb = np.full(nslot, N + 65, np.int64)  # pad -> trailing dummy
                vb[:len(selB)] = row[selB]
                rb = np.zeros(nslot, np.int64)
                rb[:len(selB)] = d[selB] - tt * 128
                brows.append((vb, rb))
            va_all = np.concatenate([x[0] for x in arows])
            vb_all = np.concatenate([x[0] for x in brows])
            ra_all = np.concatenate([x[1] for x in arows] +
                                    [x[1] for x in brows])

            def padto(v, nch, fill):
                tgt = ((nch + 1) // 2) * 2 * 128
                o = np.full(tgt, fill, np.int64)
                o[:len(v)] = v
                return o
            va_pad = padto(va_all, len(va_all) // 128, 0)
            vb_pad = padto(vb_all - ATH, len(vb_all) // 128, N + 65 - ATH)
            idxA_cols.append(wrap16_block(va_pad, len(va_pad)))
            idxB_cols.append(wrap16_block(vb_pad, len(vb_pad)))
            nch_pair = (len(va_all) + len(vb_all)) // 128
            dstrow[:, c0:c0 + nch_pair] = \
                ra_all.reshape(nch_pair, 128).T.astype(np.float32)
            dstrow_i[c0:c0 + nch_pair] = ra_all.reshape(nch_pair, 128)
            c0 += nch_pair
        idxA = np.tile(np.concatenate(idxA_cols, axis=1), (8, 1))
        idxB = np.tile(np.concatenate(idxB_cols, axis=1), (8, 1))
        # host-precomputed one-hot masks (bf16):
        #   mskT[d, c*128+s] = (dstrow[s, c] == d)   (dst-major, ed bcast)
        #   mskS[s, c*128+d] = (dstrow[s, c] == d)   (slot-major, scatter)
        mskT = np.zeros((128, NCH * 128), BF16NP)
        cc_ = np.arange(NCH * 128) // 128
        ss_ = np.arange(NCH * 128) % 128
        mskT[dstrow_i[cc_, ss_], np.arange(NCH * 128)] = 1
        mskS = np.zeros((128, NCH * 128), BF16NP)
        cols = cc_ * 128 + dstrow_i[cc_, ss_]
        mskS[ss_, cols] = 1  # row s, col c*128+dst(c, s)
        out.append({"idxA": idxA, "idxB": idxB, "dstrow": dstrow,
                    "mskT": mskT, "mskS": mskS})
    return out, pair_meta, NCH


# --------------------------------------------------------------------------
# device program
# --------------------------------------------------------------------------
def build_program(cfg, pair_meta, NCH):
    import concourse.bass as bass
    import concourse.mybir as mybir
    import concourse.tile as tile
    from concourse.library_config import mlp
    from concourse.masks import make_identity
    from concourse.tile_rust import add_dep_helper

    def _mi(x):
        return getattr(x, "ins", x)

    def dep(a, b, why):
        add_dep_helper(_mi(a), _mi(b), reason=why)

    F32 = mybir.dt.float32
    BF16 = mybir.dt.bfloat16
    I16 = mybir.dt.int16

    N, P, R, TPC = cfg["N"], cfg["P"], cfg["R"], cfg["TPC"]
    F_IN, HID, HEADS, OUT = cfg["F_IN"], cfg["HID"], cfg["HEADS"], cfg["OUT"]
    ATH = cfg["ATH"]
    HC = HID * HEADS
    IN2 = HC + F_IN
    TROW = cfg["TROW"]            # bf16 cols of big table row (h|es|pad)
    TROW3 = cfg["TROW3"]
    DGAP = 64                     # spill gap before trailing dummy
    NRT = N + DGAP + 2            # rows: dummy, N nodes, gap, dummy
    NTILE = TPC * 128

    CA = sum(m[4] for m in pair_meta)
    CB = sum(m[5] for m in pair_meta)

    nc = bass.Bass()

    ps = {}
    def par(name, shape, dt):
        ps[name] = nc.declare_dram_parameter(name, list(shape), dt,
                                             isOutput=False)
        return ps[name]

    xT = par("xT", [F_IN, R], BF16)
    Wm1 = par("Wm1", [F_IN, HC], BF16)
    Wa1 = par("Wa1", [F_IN, 2 * HEADS], BF16)
    Wm2 = par("Wm2", [IN2, HC], BF16)
    Wa2 = par("Wa2", [IN2, 2 * HEADS], BF16)
    Wm3 = par("Wm3", [IN2, OUT], BF16)
    Wa3 = par("Wa3", [IN2, 2], BF16)
    b1 = par("b1", [128, HC], F32)
    b2 = par("b2", [128, HC], F32)
    b3 = par("b3", [128, OUT], F32)
    idxA_p = par("idxA", [128, CA * 8], I16)
    idxB_p = par("idxB", [128, CB * 8], I16)
    dstrow_p = par("dstrow", [128, NCH], F32)
    mskT_p = par("mskT", [128, NCH * 128], BF16)
    iota_p = par("iota", [128, 128], F32)
    dum576 = par("dum576", [1, TROW], BF16)
    dum128 = par("dum128", [1, TROW3], BF16)
    out_ext = nc.declare_dram_parameter("out", [R, OUT], F32, isOutput=True)

    T1 = nc.dram_tensor("T1", [NRT, TROW], BF16, addr_space="Shared")
    T2 = nc.dram_tensor("T2", [NRT, TROW], BF16, addr_space="Shared")
    T3 = nc.dram_tensor("T3", [NRT, TROW3], BF16, addr_space="Shared")
    T1sh = nc.dram_tensor("T1sh", [R, TROW], BF16)
    T2sh = nc.dram_tensor("T2sh", [R, TROW], BF16)
    T3sh = nc.dram_tensor("T3sh", [R, TROW3], BF16)
    barr_i = nc.dram_tensor("barr_i", [4, 64], F32)
    barr_o = nc.dram_tensor("barr_o", [4, 64], F32, addr_space="Shared")

    nc.gpsimd.load_library(mlp)

    from contextlib import ExitStack
    _regstack = ExitStack()
    _regcache = {}

    def numreg(v):
        if v not in _regcache:
            r = _regstack.enter_context(nc.gpsimd.register(f"nidx{v}"))
            nc.gpsimd.reg_mov(r, v)
            _regcache[v] = r
        return _regcache[v]

    with tile.TileContext(nc) as tc:
        with (
            tc.tile_pool(name="const", bufs=1) as constp,
            tc.tile_pool(name="w", bufs=1) as wp,
            tc.tile_pool(name="xt", bufs=1) as xtp,
            tc.tile_pool(name="outT", bufs=1) as outTp,
            tc.tile_pool(name="ed", bufs=1) as edp,
            tc.tile_pool(name="mmz", bufs=2) as mmzp,
            tc.tile_pool(name="gA", bufs=2) as gAp,
            tc.tile_pool(name="gB", bufs=2) as gBp,
            tc.tile_pool(name="gS", bufs=2) as gSp,
            tc.tile_pool(name="seg", bufs=2) as segp,
            tc.tile_pool(name="mt", bufs=3) as mtp,
            tc.tile_pool(name="ep", bufs=2) as epp,
            tc.tile_pool(name="psA", bufs=3, space="PSUM") as psAp,
            tc.tile_pool(name="psD", bufs=2, space="PSUM") as psDp,
            tc.tile_pool(name="psE", bufs=2, space="PSUM") as psEp,
            tc.tile_pool(name="psT", bufs=1, space="PSUM") as psTp,
        ):
            # ---------- constants / resident data
            ident = constp.tile([128, 128], BF16, tag="ident")
            make_identity(nc, ident[:])
            iota_f = constp.tile([128, 128], F32, tag="iotaf")
            nc.sync.dma_start(out=iota_f[:], in_=iota_p[:])
            c02 = constp.tile([128, 1], F32, tag="c02")
            nc.vector.memset(c02[:], cfg["NEG_SLOPE"])

            idxA_sb = constp.tile([128, CA * 8], I16, tag="idxA")
            nc.sync.dma_start(out=idxA_sb[:], in_=idxA_p[:])
            idxB_sb = constp.tile([128, CB * 8], I16, tag="idxB")
            nc.sync.dma_start(out=idxB_sb[:], in_=idxB_p[:])
            dstrow_sb = constp.tile([128, NCH], F32, tag="dstrow")
            nc.sync.dma_start(out=dstrow_sb[:], in_=dstrow_p[:])

            bias_sb = {}
            for nm, p_, w_ in (("b1", b1, HC), ("b2", b2, HC), ("b3", b3, OUT)):
                bias_sb[nm] = constp.tile([128, w_], F32, tag=nm, name=nm)
                nc.sync.dma_start(out=bias_sb[nm][:], in_=p_[:])

            DB = N + DGAP + 1
            dummy_w = {}
            dummy_src = {}
            for T_, dum in ((T1, dum576), (T2, dum576), (T3, dum128)):
                i1 = nc.sync.dma_start(out=T_[0:1, :], in_=dum[:])
                i2 = nc.sync.dma_start(out=T_[DB:DB + 1, :], in_=dum[:])
                dummy_w[id(T_)] = [i1, i2]
                dummy_src[id(T_)] = dum

            def load_w(p_, rows, cols, tag):
                nchunks = (rows + 127) // 128
                tl = wp.tile([128, nchunks * cols], BF16, tag=tag, name=tag)
                for fc in range(nchunks):
                    r0 = fc * 128
                    vr = min(128, rows - r0)
                    nc.sync.dma_start(out=tl[:vr, fc * cols:(fc + 1) * cols],
                                      in_=p_[r0:r0 + vr, :])
                return tl

            Wm1_sb = load_w(Wm1, F_IN, HC, "Wm1")
            Wa1_sb = load_w(Wa1, F_IN, 2 * HEADS, "Wa1")
            Wm2_sb = load_w(Wm2, IN2, HC, "Wm2")
            Wa2_sb = load_w(Wa2, IN2, 2 * HEADS, "Wa2")
            Wm3_sb = load_w(Wm3, IN2, OUT, "Wm3")
            Wa3_sb = load_w(Wa3, IN2, 2, "Wa3")

            xT_sb = xtp.tile([128, (F_IN // 128) * R], BF16, tag="xT")
            for fc in range(F_IN // 128):
                nc.sync.dma_start(out=xT_sb[:, fc * R:(fc + 1) * R],
                                  in_=xT[fc * 128:(fc + 1) * 128, :])

            outT_sb = outTp.tile([128, (HC // 128) * NTILE], BF16, tag="outT")
            # per-layer local ed tiles: bf16 (matmul rhs) + f32 (self chunk)
            edb_sb = edp.tile([128, TPC * 8], BF16, tag="edb")
            edf_sb = edp.tile([128, TPC * 8], F32, tag="edf")

            # ---------- matmul phase (one tile)
            def mm_tile(layer, rt, tsh_w):
                if layer == 1:
                    nfc, Wm_sb, Wa_sb, Tsh, trow, hcols, nh = (
                        F_IN // 128, Wm1_sb, Wa1_sb, T1sh, TROW, HC, HEADS)
                elif layer == 2:
                    nfc, Wm_sb, Wa_sb, Tsh, trow, hcols, nh = (
                        IN2 // 128, Wm2_sb, Wa2_sb, T2sh, TROW, HC, HEADS)
                else:
                    nfc, Wm_sb, Wa_sb, Tsh, trow, hcols, nh = (
                        IN2 // 128, Wm3_sb, Wa3_sb, T3sh, TROW3, OUT, 1)
                acols = 2 * nh
                nxc = HC // 128

                r0 = rt * 128
                vr = min(128, R - r0)
                if vr <= 0:
                    return
                pm = psAp.tile([128, max(hcols, 8)], F32, tag="agg",
                               name="pm")
                pa = psDp.tile([128, 16], F32, tag="den", name="pa")
                for fc in range(nfc):
                    if layer == 1:
                        lhsT = xT_sb[:, fc * R + r0: fc * R + r0 + vr]
                    elif fc < nxc:
                        lhsT = outT_sb[:, fc * NTILE + r0:
                                       fc * NTILE + r0 + vr]
                    else:
                        fx = fc - nxc
                        lhsT = xT_sb[:, fx * R + r0: fx * R + r0 + vr]
                    nc.tensor.matmul(out=pm[:vr, :hcols], lhsT=lhsT,
                                     rhs=Wm_sb[:, fc * hcols:(fc + 1) * hcols],
                                     start=(fc == 0), stop=(fc == nfc - 1))
                    nc.tensor.matmul(out=pa[:vr, :acols], lhsT=lhsT,
                                     rhs=Wa_sb[:, fc * acols:(fc + 1) * acols],
                                     start=(fc == 0), stop=(fc == nfc - 1))
                zrow = mmzp.tile([128, TROW], BF16, tag="zrow")
                nc.vector.memset(zrow[:, hcols + 2 * nh:trow], 0.0)
                nc.vector.tensor_copy(out=zrow[:vr, :hcols],
                                      in_=pm[:vr, :hcols])
                nc.vector.tensor_copy(
                    out=zrow[:vr, hcols:hcols + 2 * nh].bitcast(F32),
                    in_=pa[:vr, 0:nh])
                nc.vector.tensor_copy(out=edb_sb[:vr, rt * 8:rt * 8 + nh],
                                      in_=pa[:vr, nh:2 * nh])
                nc.vector.tensor_copy(out=edf_sb[:vr, rt * 8:rt * 8 + nh],
                                      in_=pa[:vr, nh:2 * nh])
                tsh_w.append(nc.sync.dma_start(out=Tsh[r0:r0 + vr, :],
                                               in_=zrow[:vr, :trow]))

            # per-layer chunked AllGather state
            def ag_chunk(layer, k, Tsh_, T_, tsh_w, ccs):
                bnds = cfg["AG_BNDS"]
                base = cfg["AG_BASE"]
                ta, tb = bnds[k], bnds[k + 1]
                if tb <= ta:
                    return
                ra, rb = ta * 128, min(tb * 128, R)
                o0 = 1 + int(base[k])
                cc = nc.gpsimd.collective_compute(
                    "AllGather",
                    mybir.AluOpType.bypass,
                    replica_groups=[list(range(P))],
                    ins=[Tsh_[ra:rb, :]],
                    outs=[T_[o0:o0 + P * (rb - ra), :]],
                )
                for t in range(ta, tb):
                    dep(cc, tsh_w[t], "AG chunk reads shard rows")
                ccs.append(cc)

            last_gather = [None]

            # ---------- aggregation phase
            def agg_phase(layer, ccs, Tsh_, tsh_w, after_tile=None):
                pass_T = {1: T1, 2: T2, 3: T3}[layer]
                # refresh dummy rows after AG chunks (guards against any
                # collective overrun clobbering the trailing dummy row)
                dre = nc.sync.dma_start(out=pass_T[DB:DB + 1, :],
                                        in_=dummy_src[id(pass_T)][:])
                for cc in ccs:
                    dep(dre, cc, "dummy refresh after AG chunks")
                dummy_w[id(pass_T)] = dummy_w[id(pass_T)] + [dre]
                if layer == 3:
                    T_, trow, hcols, nh = T3, TROW3, OUT, 1
                    bias = bias_sb["b3"]
                else:
                    T_, trow, hcols, nh = (T1 if layer == 1 else T2), TROW, HC, HEADS
                    bias = bias_sb["b1"] if layer == 1 else bias_sb["b2"]

                offA = offB = 0
                for (tiles, aseg, bseg, c0, napad, nbpad) in pair_meta:
                    ntl = len(tiles)
                    bufA = gAp.tile([128, cfg["MAXA"] * trow], BF16, tag="bufA")
                    bufB = gBp.tile([128, cfg["MAXB"] * trow], BF16, tag="bufB")
                    SUBG = 8
                    gAs, gBs = [], []
                    for cs in range(0, napad, SUBG):
                        ck = min(SUBG, napad - cs)
                        gAs.append(nc.gpsimd.dma_gather(
                            bufA[:, cs * trow:(cs + ck) * trow].rearrange(
                                "p (c w) -> p c w", w=trow),
                            T_[:], idxA_sb[:, offA + cs * 8:offA + (cs + ck) * 8],
                            ck * 128, numreg(ck * 128), trow))
                    for cs in range(0, nbpad, SUBG):
                        ck = min(SUBG, nbpad - cs)
                        gBs.append(nc.gpsimd.dma_gather(
                            bufB[:, cs * trow:(cs + ck) * trow].rearrange(
                                "p (c w) -> p c w", w=trow),
                            T_[ATH:, :],
                            idxB_sb[:, offB + cs * 8:offB + (cs + ck) * 8],
                            ck * 128, numreg(ck * 128), trow))
                    for g_ in gAs + gBs:
                        for cc in ccs:
                            dep(g_, cc, "gather reads allgathered table")
                        for d_ in dummy_w[id(T_)]:
                            dep(g_, d_, "gather reads dummy rows")
                        last_gather[0] = g_
                    offA += napad * 8; offB += nbpad * 8

                    # self rows (own shard, sequential read; no AG dep)
                    selfr = gSp.tile([128, 2 * trow], BF16, tag="selfr")
                    for ti, tt in enumerate(tiles):
                        r0l = tt * 128
                        vr = min(128, R - r0l)
                        sr = nc.sync.dma_start(
                            out=selfr[:vr, ti * trow:(ti + 1) * trow],
                            in_=Tsh_[r0l:r0l + vr, :])
                        dep(sr, tsh_w[tt], "self rows read own shard write")

                    # psum accumulators per tile
                    pag = [psAp.tile([128, max(hcols, 8)], F32, tag="agg",
                                     name=f"pag{_i}") for _i in range(ntl)]
                    pde = [psDp.tile([128, 8], F32, tag="den",
                                     name=f"pde{_i}") for _i in range(ntl)]
                    started = [False] * ntl

                    # segments: (tile_i, buf, bufc0, nchunks)
                    segs = []
                    bc = 0
                    for ti in range(ntl):
                        segs.append((ti, bufA, bc, aseg[ti]))
                        bc += aseg[ti]
                    bc = 0
                    for ti in range(ntl):
                        segs.append((ti, bufB, bc, bseg[ti]))
                        bc += bseg[ti]

                    pc = 0
                    for (ti, buf, bc0, nck) in segs:
                        if nck == 0:
                            continue
                        bv = buf[:, : (bc0 + nck) * trow].rearrange(
                            "p (c w) -> p c w", w=trow)
                        # one-hot dst masks for the segment
                        msk = segp.tile([128, cfg["MAXSEG"] * 128], BF16,
                                        tag="msk")
                        nc.vector.tensor_tensor(
                            out=msk[:, :nck * 128].rearrange(
                                "p (c r) -> p c r", c=nck),
                            in0=dstrow_sb[:, c0 + pc:c0 + pc + nck, None]
                                .to_broadcast([128, nck, 128]),
                            in1=iota_f[:, None, :]
                                .to_broadcast([128, nck, 128]),
                            op=mybir.AluOpType.is_equal)
                        # ed broadcast matmul with host-precomputed mskT
                        mtT = mtp.tile([128, cfg["MAXSEG"] * 128], BF16,
                                       tag="mtT", name="mtT")
                        nc.scalar.dma_start(
                            out=mtT[:, :nck * 128],
                            in_=mskT_p[:, (c0 + pc) * 128:
                                       (c0 + pc + nck) * 128])
                        pse = psEp.tile([128, cfg["MAXSEG"] * 8], F32,
                                        tag="pse", name="pse")
                        for j in range(nck):
                            nc.tensor.matmul(
                                out=pse[:, j * 8:j * 8 + nh],
                                lhsT=mtT[:, j * 128:(j + 1) * 128],
                                rhs=edb_sb[:, tiles[ti] * 8:tiles[ti] * 8 + nh],
                                start=True, stop=True,
                                skip_group_check=True)
                        # logits
                        es_ap = bv[:, bc0:bc0 + nck,
                                   hcols:hcols + 2 * nh].bitcast(F32)
                        et = segp.tile([128, cfg["MAXSEG"] * 8], F32, tag="et")
                        nc.vector.tensor_tensor(
                            out=et[:, :nck * 8].rearrange(
                                "p (c h) -> p c h", h=8)[:, :, :nh],
                            in0=es_ap,
                            in1=pse[:, :nck * 8].rearrange(
                                "p (c h) -> p c h", h=8)[:, :, :nh],
                            op=mybir.AluOpType.add)
                        etl = segp.tile([128, cfg["MAXSEG"] * 8], F32,
                                        tag="etl")
                        def _v8(t):
                            return t[:, :nck * 8].rearrange(
                                "p (c h) -> p c h", h=8)[:, :, :nh]
                        nc.vector.tensor_tensor(
                            out=_v8(etl), in0=_v8(et),
                            in1=c02[:, 0:1, None].to_broadcast([128, nck, nh]),
                            op=mybir.AluOpType.mult)
                        nc.vector.tensor_tensor(
                            out=_v8(et), in0=_v8(et),
                            in1=_v8(etl), op=mybir.AluOpType.max)
                        ex = segp.tile([128, cfg["MAXSEG"] * 8], BF16,
                                       tag="ex")
                        nc.scalar.activation(
                            out=_v8(ex), in_=_v8(et),
                            func=mybir.ActivationFunctionType.Exp)
                        # scaled messages
                        mp_ = segp.tile([128, cfg["MAXSEG"] * hcols], BF16,
                                        tag="mp")
                        nc.vector.tensor_tensor(
                            out=mp_[:, :nck * hcols].rearrange(
                                "p (c h k) -> p c h k", c=nck, h=nh),
                            in0=bv[:, bc0:bc0 + nck, 0:hcols].rearrange(
                                "p c (h k) -> p c h k", h=nh),
                            in1=ex[:, :nck * 8].rearrange(
                                "p (c h) -> p c h", h=8)[:, :, :nh, None]
                                .to_broadcast([128, nck, nh, hcols // nh]),
                            op=mybir.AluOpType.mult)
                        for j in range(nck):
                            nc.tensor.matmul(
                                out=pag[ti][:, :hcols],
                                lhsT=msk[:, j * 128:(j + 1) * 128],
                                rhs=mp_[:, j * hcols:(j + 1) * hcols],
                                start=not started[ti], stop=False,
                                skip_group_check=True)
                            nc.tensor.matmul(
                                out=pde[ti][:, :nh],
                                lhsT=msk[:, j * 128:(j + 1) * 128],
                                rhs=ex[:, j * 8:j * 8 + nh],
                                start=not started[ti], stop=False,
                                skip_group_check=True)
                            started[ti] = True
                        pc += nck

                    # self chunk + epilogue per tile
                    for ti, tt in enumerate(tiles):
                        vr = min(128, R - tt * 128)
                        sv = selfr[:, ti * trow:(ti + 1) * trow]
                        es_s = sv[:, hcols:hcols + 2 * nh].bitcast(F32)
                        ets = mtp.tile([128, 8], F32, tag="ets", name="ets")
                        nc.vector.tensor_tensor(
                            out=ets[:, :nh], in0=es_s,
                            in1=edf_sb[:, tt * 8:tt * 8 + nh],
                            op=mybir.AluOpType.add)
                        etls = mtp.tile([128, 8], F32, tag="etls",
                                        name="etls")
                        nc.vector.tensor_tensor(
                            out=etls[:, :nh], in0=ets[:, :nh],
                            in1=c02[:, 0:1].to_broadcast([128, nh]),
                            op=mybir.AluOpType.mult)
                        nc.vector.tensor_tensor(
                            out=ets[:, :nh], in0=ets[:, :nh],
                            in1=etls[:, :nh], op=mybir.AluOpType.max)
                        exs = mtp.tile([128, 8], BF16, tag="exs", name="exs")
                        nc.scalar.activation(
                            out=exs[:, :nh], in_=ets[:, :nh],
                            func=mybir.ActivationFunctionType.Exp)
                        mps = mtp.tile([128, hcols], BF16, tag="mps",
                                       name="mps")
                        nc.vector.tensor_tensor(
                            out=mps[:].rearrange("p (h k) -> p h k", h=nh),
                            in0=sv[:, 0:hcols].rearrange(
                                "p (h k) -> p h k", h=nh),
                            in1=exs[:, :nh, None].to_broadcast(
                                [128, nh, hcols // nh]),
                            op=mybir.AluOpType.mult)
                        nc.tensor.matmul(
                            out=pag[ti][:, :hcols], lhsT=ident[:],
                            rhs=mps[:], start=not started[ti], stop=True,
                            skip_group_check=True)
                        nc.tensor.matmul(
                            out=pde[ti][:, :nh], lhsT=ident[:],
                            rhs=exs[:, :nh], start=not started[ti],
                            stop=True, skip_group_check=True)

                        den = epp.tile([128, 8], F32, tag="den_sb")
                        nc.vector.reciprocal(out=den[:, :nh],
                                             in_=pde[ti][:, :nh])
                        o1 = epp.tile([128, max(hcols, 8)], F32, tag="o1")
                        nc.vector.tensor_tensor(
                            out=o1[:, :hcols].rearrange(
                                "p (h k) -> p h k", h=nh),
                            in0=pag[ti][:, :hcols].rearrange(
                                "p (h k) -> p h k", h=nh),
                            in1=den[:, :nh, None].to_broadcast(
                                [128, nh, hcols // nh]),
                            op=mybir.AluOpType.mult)
                        nc.vector.tensor_tensor(
                            out=o1[:, :hcols], in0=o1[:, :hcols],
                            in1=bias[:, :],
                            op=mybir.AluOpType.add)
                        r0 = tt * 128
                        if layer != 3:
                            ob = epp.tile([128, hcols], BF16, tag="ob")
                            nc.scalar.activation(
                                out=ob[:, :], in_=o1[:, :hcols],
                                func=mybir.ActivationFunctionType.Relu)
                            for q in range(hcols // 128):
                                pt2 = psTp.tile([128, 128], BF16, tag="pt2",
                                                name="pt2")
                                nc.tensor.transpose(
                                    out=pt2[:, :vr],
                                    in_=ob[:vr, q * 128:(q + 1) * 128],
                                    identity=ident[:vr, :vr])
                                nc.vector.tensor_copy(
                                    out=outT_sb[:, q * NTILE + r0:
                                                q * NTILE + r0 + vr],
                                    in_=pt2[:, :vr])
                        else:
                            mx = epp.tile([128, 1], F32, tag="mx")
                            nc.vector.tensor_reduce(
                                out=mx[:], in_=o1[:, :hcols],
                                op=mybir.AluOpType.max,
                                axis=mybir.AxisListType.X)
                            zc = epp.tile([128, hcols], F32, tag="zc")
                            nc.vector.tensor_scalar(
                                out=zc[:], in0=o1[:, :hcols], scalar1=mx[:],
                                scalar2=None,
                                op0=mybir.AluOpType.subtract)
                            ex3 = epp.tile([128, hcols], F32, tag="ex3")
                            s3 = epp.tile([128, 1], F32, tag="s3")
                            nc.scalar.activation(
                                out=ex3[:], in_=zc[:],
                                func=mybir.ActivationFunctionType.Exp,
                                accum_out=s3[:])
                            ln3 = epp.tile([128, 1], F32, tag="ln3")
                            nc.scalar.activation(
                                out=ln3[:], in_=s3[:],
                                func=mybir.ActivationFunctionType.Ln)
                            res = epp.tile([128, hcols], F32, tag="res")
                            nc.vector.tensor_scalar(
                                out=res[:], in0=zc[:], scalar1=ln3[:],
                                scalar2=None,
                                op0=mybir.AluOpType.subtract)
                            nc.sync.dma_start(out=out_ext[r0:r0 + vr, :],
                                              in_=res[:vr, :])
                        if after_tile is not None:
                            after_tile(tt)

            # ---------- the three layers
            # layer k+1's matmul tiles are emitted inside layer k's
            # aggregation (after each tile's epilogue) so the tensor work
            # overlaps the gather-bound agg phase.  L1's AllGather is
            # chunked to overlap L1's matmul phase; L2/L3 AllGathers are
            # monolithic and serialize after the previous agg (their tsh
            # deps transitively follow every gather of the previous
            # layer), keeping collective DMA and gather DMA disjoint.
            bnds = cfg["AG_BNDS"]
            chunk_end = {bnds[k + 1] - 1: k for k in range(len(bnds) - 1)
                         if bnds[k + 1] > bnds[k]}
            tshs = {1: [], 2: [], 3: []}
            ccss = {1: [], 2: [], 3: []}
            tabs = {1: (T1sh, T1), 2: (T2sh, T2), 3: (T3sh, T3)}

            for rt in range(TPC):
                mm_tile(1, rt, tshs[1])
                if rt in chunk_end:
                    ag_chunk(1, chunk_end[rt], T1sh, T1, tshs[1], ccss[1])

            def mk_after(nl):
                def after(tt):
                    mm_tile(nl, tt, tshs[nl])
                return after

            def ag_serial(nl):
                # chunk-major AG chunks, serialized after the previous
                # layer's last gather so CC DMA never overlaps gather DMA
                Tsh_n, T_n = tabs[nl]
                lg = last_gather[0]
                for k in range(len(bnds) - 1):
                    if bnds[k + 1] <= bnds[k]:
                        continue
                    ag_chunk(nl, k, Tsh_n, T_n, tshs[nl], ccss[nl])
                    if lg is not None:
                        dep(ccss[nl][-1], lg,
                            "AG serialized after prev-layer gathers")

            def standalone(nl):
                for rt in range(TPC):
                    mm_tile(nl, rt, tshs[nl])
                    if rt in chunk_end:
                        ag_chunk(nl, chunk_end[rt], tabs[nl][0], tabs[nl][1],
                                 tshs[nl], ccss[nl])

            agg_phase(1, ccss[1], T1sh, tshs[1])
            standalone(2)
            agg_phase(2, ccss[2], T2sh, tshs[2])
            standalone(3)
            agg_phase(3, ccss[3], T3sh, tshs[3])

    _regstack.close()
    from concourse.library_overlay import lower_extended_insts
    lower_extended_insts(nc)
    return nc


# --------------------------------------------------------------------------
# host wrapper
# --------------------------------------------------------------------------
def _prep_inputs(inputs, cfg):
    N, P, R, TPC = cfg["N"], cfg["P"], cfg["R"], cfg["TPC"]
    HEADS, HID, OUT, F_IN = cfg["HEADS"], cfg["HID"], cfg["OUT"], cfg["F_IN"]
    HC = HEADS * HID

    x = np.asarray(inputs["x"], np.float32)
    edge_index = np.asarray(inputs["edge_index"], np.int64)

    nchunks = min(6, TPC)
    bnds = [TPC * k // nchunks for k in range(nchunks + 1)]
    perm, base, _rows_k = table_perm(N, P, R, bnds)
    cfg["AG_BNDS"] = bnds
    cfg["AG_BASE"] = base
    shards, pair_meta, NCH = build_edges(edge_index, N, P, R, TPC, cfg["ATH"],
                                         perm)

    def fold(W, a_s, a_d, heads, ch):
        F = W.shape[0]
        Wr = W.reshape(F, heads, ch)
        Wa = np.zeros((F, 2 * heads), np.float32)
        for h in range(heads):
            Wa[:, h] = Wr[:, h] @ a_s[h]
            Wa[:, heads + h] = Wr[:, h] @ a_d[h]
        return Wa

    w1 = np.asarray(inputs["w1"], np.float32)
    w2 = np.asarray(inputs["w2"], np.float32)
    w3 = np.asarray(inputs["w3"], np.float32)
    Wa1 = fold(w1, np.asarray(inputs["a1s"]), np.asarray(inputs["a1d"]),
               HEADS, HID)
    Wa2 = fold(w2, np.asarray(inputs["a2s"]), np.asarray(inputs["a2d"]),
               HEADS, HID)
    Wa3 = fold(w3, np.asarray(inputs["a3s"]), np.asarray(inputs["a3d"]),
               1, OUT)

    dum576 = np.zeros((1, cfg["TROW"]), BF16NP)
    dum576.view(np.uint8)[0, 2 * HC:2 * HC + HEADS * 4] = \
        np.full(HEADS, NEG_BIG, np.float32).view(np.uint8)
    dum128 = np.zeros((1, cfg["TROW3"]), BF16NP)
    dum128.view(np.uint8)[0, 2 * OUT:2 * OUT + 4] = \
        np.frombuffer(np.float32(NEG_BIG).tobytes(), np.uint8)

    common = {
        "Wm1": w1.astype(BF16NP), "Wa1": Wa1.astype(BF16NP),
        "Wm2": w2.astype(BF16NP), "Wa2": Wa2.astype(BF16NP),
        "Wm3": w3.astype(BF16NP), "Wa3": Wa3.astype(BF16NP),
        "b1": np.tile(np.asarray(inputs["b1"], np.float32).reshape(1, HC),
                      (128, 1)),
        "b2": np.tile(np.asarray(inputs["b2"], np.float32).reshape(1, HC),
                      (128, 1)),
        "b3": np.tile(np.asarray(inputs["b3"], np.float32).reshape(1, OUT),
                      (128, 1)),
        "dum576": dum576, "dum128": dum128,
        "iota": np.tile(np.arange(128, dtype=np.float32), (128, 1)),
    }
    in_maps = []
    for p in range(P):
        m = dict(common)
        m["xT"] = np.ascontiguousarray(
            x[p * R:(p + 1) * R, :].T).astype(BF16NP)
        m["idxA"] = shards[p]["idxA"]
        m["idxB"] = shards[p]["idxB"]
        m["dstrow"] = shards[p]["dstrow"]
        m["mskT"] = shards[p]["mskT"]
        in_maps.append(m)

    maxa = max(m[4] for m in pair_meta)
    maxb = max(m[5] for m in pair_meta)
    cfg["MAXA"], cfg["MAXB"] = maxa, maxb
    cfg["MAXSEG"] = max(max(a + b) for (_, a, b, _c, _1, _2) in pair_meta)
    return in_maps, pair_meta, NCH


def default_cfg():
    return dict(N=50000, P=8, R=6250, TPC=49, F_IN=256, HID=64, HEADS=8,
                OUT=16, ATH=32768, TROW=640, TROW3=128, NEG_SLOPE=0.2)


def run_with_cfg(cfg, inputs):
    in_maps, pair_meta, NCH = _prep_inputs(inputs, cfg)
    nc = build_program(cfg, pair_meta, NCH)

    _split_multiwait(nc)
    from concourse.bass_utils import run_bass_kernel_spmd
    trace = bool(os.environ.get("GNN_TRACE"))
    if trace:
        sys.path.insert(0, "/root/problem/work")
        import axonhook  # noqa
    res = run_bass_kernel_spmd(nc, in_maps, list(range(cfg["P"])),
                               trace=trace)
    if trace:
        kernel.last_exec_ns = res.exec_time_ns
    out = np.concatenate([res.results[p]["out"] for p in range(cfg["P"])],
                         axis=0)
    return out.astype(np.float32)


def kernel(**inputs):
    return run_with_cfg(default_cfg(), inputs)
